# revision 9
# baseline (speedup 1.0000x reference)
"""MoE (top-2, 8 experts, capacity-factor 1.0) Trainium2 Bass kernel.

Token-parallel across 8 NeuronCores: each core owns a 4096-token shard and
runs router (fp32) -> top-2 -> prefix-sum compaction -> dma_gather dispatch ->
per-expert MLP (bf16 matmuls, fp32 accumulate) -> gated dma_scatter_add
combine, entirely on-device.  Aux-loss partials are reduced on host (8x8
scalars).  Self-contained: hardcodes N=32768, D=512, H=1024, E=8, K=2.
"""

import numpy as np

import concourse.bass as bass
import concourse.bacc as bacc
import concourse.mybir as mybir
import concourse.tile as tile
from concourse.bass_utils import run_bass_kernel_spmd
from concourse.tile_rust import add_dep_helper

dt = mybir.dt
Alu = mybir.AluOpType
ACTF = mybir.ActivationFunctionType
AX = mybir.AxisListType

N_CORES = 8
N_TOK, D, H, E = 32768, 512, 1024, 8
NT = N_TOK // N_CORES          # 4096 tokens per core
NBI = NT // 128                # 32
CAP = 1280                     # per-(core, expert) slot capacity
NTILE = CAP // 128             # 10
PAY = 64                       # table payload in fp32 elems (256 B)
TROWS = 1408
GROUPS = [(0, 4), (4, 8), (8, 10)]

_COMPILED = None


def _build():
    nc = bacc.Bacc("TRN2", target_bir_lowering=False, debug=False,
                   num_devices=N_CORES, enable_asserts=False)

    xT_in = nc.declare_dram_parameter("xT_in", [513, NT], dt.float32, isOutput=False)
    x_in = nc.declare_dram_parameter("x_in", [NT + 1, D], dt.bfloat16, isOutput=False)
    rw_in = nc.declare_dram_parameter("rw_in", [513, 8], dt.float32, isOutput=False)
    w1_in = nc.declare_dram_parameter("w1_in", [E, D, H], dt.bfloat16, isOutput=False)
    w2_in = nc.declare_dram_parameter("w2_in", [E, H, D], dt.bfloat16, isOutput=False)
    b1_in = nc.declare_dram_parameter("b1_in", [E, H], dt.float32, isOutput=False)
    b2_in = nc.declare_dram_parameter("b2_in", [E, D], dt.bfloat16, isOutput=False)
    iota8_c = nc.declare_dram_parameter("iota8_c", [128, NBI * 8], dt.float32, isOutput=False)
    idxc_c = nc.declare_dram_parameter("idxc_c", [128, NBI], dt.float32, isOutput=False)
    lt_c = nc.declare_dram_parameter("lt_c", [128, 128], dt.float32, isOutput=False)
    id_c = nc.declare_dram_parameter("id_c", [128, 128], dt.float32, isOutput=False)
    ones_c = nc.declare_dram_parameter("ones_c", [128, 1], dt.float32, isOutput=False)

    o_out = nc.declare_dram_parameter("o_out", [NT + 1, D], dt.float32, isOutput=True)
    o_aux = nc.declare_dram_parameter("o_aux", [8, 1], dt.float32, isOutput=True)
    o_cnt = nc.declare_dram_parameter("o_cnt", [1, 8], dt.float32, isOutput=True)
    import os as _os
    _DBG = _os.environ.get("KDBG") == "1"
    if _DBG:
        o_dtab = nc.declare_dram_parameter("o_dtab", [TROWS, PAY], dt.float32, isOutput=True)
        o_dslw = nc.declare_dram_parameter("o_dslw", [16, 256], dt.float32, isOutput=True)
        o_didx = nc.declare_dram_parameter("o_didx", [128, 80], dt.float32, isOutput=True)

    bf = dt.bfloat16

    with tile.TileContext(nc) as tc:
        with tc.tile_pool(name="const", bufs=1) as cp, \
             tc.tile_pool(name="route", bufs=1) as rp, \
             tc.tile_pool(name="ps_h", bufs=2, space="PSUM") as ps_h, \
             tc.tile_pool(name="ps_o", bufs=2, space="PSUM") as ps_o, \
             tc.tile_pool(name="ps_tr", bufs=2, space="PSUM") as ps_tr, \
             tc.tile_pool(name="dram", bufs=1, space="DRAM") as dr:

            # ---------- constants ----------
            iota8 = cp.tile([128, NBI * 8], dt.float32)
            idxc = cp.tile([128, NBI], dt.float32)
            ltm = cp.tile([128, 128], dt.float32)
            idm = cp.tile([128, 128], dt.float32)
            ones1 = cp.tile([128, 1], dt.float32)
            ones_r = cp.tile([1, 128], bf)
            nc.vector.memset(ones_r[:], 1.0)
            idm_bf = cp.tile([128, 128], bf)
            nc.sync.dma_start(out=iota8[:], in_=iota8_c[:])
            nc.sync.dma_start(out=idxc[:], in_=idxc_c[:])
            nc.sync.dma_start(out=ltm[:], in_=lt_c[:])
            nc.sync.dma_start(out=idm[:], in_=id_c[:])
            nc.vector.tensor_copy(idm_bf[:], idm[:])
            nc.sync.dma_start(out=ones1[:], in_=ones_c[:])
            rw_sb = cp.tile([128, 4, 8], dt.float32)
            nc.sync.dma_start(
                out=rw_sb[:],
                in_=bass.AP(rw_in[:].tensor, 0, [[8, 128], [128 * 8, 4], [1, 8]]))
            rb_row = cp.tile([1, 8], dt.float32)
            nc.sync.dma_start(out=rb_row[:], in_=rw_in[512:513, :])

            logit = rp.tile([128, NBI, 8], dt.float32)

            # ---------- router (own pool; closes to free 64 KB/part) ----------
            with tc.tile_pool(name="xt", bufs=1) as xp:
                xT_sb = xp.tile([128, 4, NT], dt.float32)
                nc.sync.dma_start(
                    out=xT_sb[:],
                    in_=bass.AP(xT_in[:].tensor, 0, [[NT, 128], [128 * NT, 4], [1, NT]]))
                ones_row = xp.tile([1, NT], dt.float32)
                nc.sync.dma_start(out=ones_row[:], in_=xT_in[512:513, :])
                for g in range(NBI):
                    lp = ps_h.tile([128, 512], dt.float32, space="PSUM", tag="hp")
                    for kd in range(4):
                        nc.tensor.matmul(
                            lp[:, 0:8], lhsT=xT_sb[:, kd, 128 * g:128 * (g + 1)],
                            rhs=rw_sb[:, kd, :], start=(kd == 0), stop=False)
                    nc.tensor.matmul(
                        lp[:, 0:8], lhsT=ones_row[:, 128 * g:128 * (g + 1)],
                        rhs=rb_row[:], start=False, stop=True)
                    nc.scalar.activation(logit[:, g, :], lp[:, 0:8], ACTF.Copy)

            # ---------- softmax ----------
            probs = rp.tile([128, NBI, 8], dt.float32)
            ssum = rp.tile([128, NBI], dt.float32)
            srec = rp.tile([128, NBI], dt.float32)
            nc.scalar.activation(probs[:], logit[:], ACTF.Exp)
            nc.vector.tensor_reduce(ssum[:], probs[:], axis=AX.X, op=Alu.add)
            nc.vector.reciprocal(srec[:], ssum[:])
            nc.vector.tensor_tensor(out=probs[:], in0=probs[:],
                                    in1=srec[:, :, None].to_broadcast([128, NBI, 8]),
                                    op=Alu.mult)

            # ---------- top-2 ----------
            m1 = rp.tile([128, NBI], dt.float32)
            m2 = rp.tile([128, NBI], dt.float32)
            a1 = rp.tile([128, NBI], dt.float32)
            a2 = rp.tile([128, NBI], dt.float32)
            eqs = rp.tile([128, NBI, 8], dt.float32)
            tmp = rp.tile([128, NBI, 8], dt.float32)
            pm = rp.tile([128, NBI, 8], dt.float32)
            iov = iota8[:].rearrange("p (a b) -> p a b", b=8)
            nc.vector.tensor_reduce(m1[:], probs[:], axis=AX.X, op=Alu.max)
            nc.vector.tensor_tensor(out=eqs[:], in0=probs[:],
                                    in1=m1[:, :, None].to_broadcast([128, NBI, 8]),
                                    op=Alu.is_equal)
            nc.vector.tensor_tensor(out=tmp[:], in0=eqs[:], in1=iov, op=Alu.mult)
            nc.vector.tensor_reduce(a1[:], tmp[:], axis=AX.X, op=Alu.add)
            nc.vector.scalar_tensor_tensor(out=pm[:], in0=eqs[:], scalar=-1e9,
                                           in1=probs[:], op0=Alu.mult, op1=Alu.add)
            nc.vector.tensor_reduce(m2[:], pm[:], axis=AX.X, op=Alu.max)
            nc.vector.tensor_tensor(out=eqs[:], in0=pm[:],
                                    in1=m2[:, :, None].to_broadcast([128, NBI, 8]),
                                    op=Alu.is_equal)
            nc.vector.tensor_tensor(out=tmp[:], in0=eqs[:], in1=iov, op=Alu.mult)
            nc.vector.tensor_reduce(a2[:], tmp[:], axis=AX.X, op=Alu.add)

            # ---------- aux partial S_e ----------
            psum_pe = rp.tile([128, 8], dt.float32)
            nc.vector.tensor_reduce(
                psum_pe[:],
                bass.AP(probs[:].tensor, probs[:].offset,
                        [probs[:].ap[0], [1, 8], [8, NBI]]),
                axis=AX.X, op=Alu.add)
            aux_ps = ps_h.tile([128, 512], dt.float32, space="PSUM", tag="hp")
            nc.tensor.matmul(aux_ps[0:8, 0:1], lhsT=psum_pe[:], rhs=ones1[:],
                             start=True, stop=True)
            aux_sb = rp.tile([8, 1], dt.float32)
            nc.vector.tensor_copy(aux_sb[:], aux_ps[0:8, 0:1])
            nc.sync.dma_start(out=o_aux[:], in_=aux_sb[:])

            counts_f = rp.tile([1, 8], dt.float32)

            # ---------- zero output ----------
            zer = rp.tile([128, 2048], dt.float32)
            nc.vector.memset(zer[:], 0.0)
            zero_insts = []
            for c in range(8):
                zv = bass.AP(o_out[:].tensor, c * 128 * 2048, [[2048, 128], [1, 2048]])
                zero_insts.append(nc.sync.dma_start(out=zv, in_=zer[:]))
            zero_insts.append(nc.sync.dma_start(out=o_out[NT:NT + 1, :], in_=zer[0:1, 0:D]))

            # ---------- dispatch + expert compute ----------
            with tc.tile_pool(name="disp", bufs=2) as dp, \
                 tc.tile_pool(name="wts", bufs=2) as wp, \
                 tc.tile_pool(name="act", bufs=2) as ap_:

                wpays = []
                for i in range(2):
                    t = dp.tile([128, NBI, PAY], dt.float32, name=f"wpay{i}",
                                tag=f"wpay{i}", bufs=1)
                    nc.vector.memset(t[:], 0.0)
                    nc.vector.tensor_copy(t[:, :, 0], idxc[:])
                    wpays.append(t)
                tinit = rp.tile([128, (TROWS // 128) * PAY], dt.float32)
                nc.vector.memset(tinit[:], 0.0)
                tables = [dr.tile([TROWS, PAY], dt.float32, name=f"tab{i}",
                                  tag=f"tab{i}", bufs=1) for i in range(2)]
                prev_tab_users = [[], []]
                prev_scatter = []

                for e in range(E):
                    # mask + prefix-sum positions
                    mask = dp.tile([128, NBI], dt.float32, tag="mask")
                    t1 = dp.tile([128, NBI], dt.float32, tag="t1")
                    nc.vector.tensor_scalar(out=mask[:], in0=a1[:], scalar1=float(e),
                                            scalar2=None, op0=Alu.is_equal)
                    nc.vector.tensor_scalar(out=t1[:], in0=a2[:], scalar1=float(e),
                                            scalar2=None, op0=Alu.is_equal)
                    nc.vector.tensor_tensor(out=mask[:], in0=mask[:], in1=t1[:], op=Alu.add)
                    incl = dp.tile([128, NBI], dt.float32, tag="incl")
                    nc.vector.tensor_tensor_scan(out=incl[:], data0=mask[:], data1=mask[:],
                                                 initial=0.0, op0=Alu.add, op1=Alu.bypass)
                    excl = dp.tile([128, NBI], dt.float32, tag="excl")
                    nc.vector.tensor_tensor(out=excl[:], in0=incl[:], in1=mask[:],
                                            op=Alu.subtract)
                    rowp = ps_o.tile([128, D], dt.float32, space="PSUM", tag="op")
                    nc.tensor.matmul(rowp[:, 0:1], lhsT=ltm[:], rhs=incl[:, NBI - 1:NBI],
                                     start=True, stop=True)
                    nc.tensor.matmul(rowp[0:1, 1:2], lhsT=incl[:, NBI - 1:NBI], rhs=ones1[:],
                                     start=True, stop=True)
                    nc.vector.tensor_copy(counts_f[:, e:e + 1], rowp[0:1, 1:2])

                    pos2 = dp.tile([128, NBI], dt.float32, tag="pos2")
                    nc.vector.tensor_scalar(out=pos2[:], in0=excl[:], scalar1=rowp[:, 0:1],
                                            scalar2=-float(CAP), op0=Alu.add, op1=Alu.add)
                    mt = dp.tile([128, NBI], dt.float32, tag="mt")
                    nc.vector.tensor_tensor(out=mt[:], in0=mask[:], in1=pos2[:], op=Alu.mult)
                    slotf = dp.tile([128, NBI], dt.float32, tag="slotf")
                    nc.vector.tensor_scalar(out=slotf[:], in0=mt[:], scalar1=float(CAP),
                                            scalar2=None, op0=Alu.add)

                    # wrap slotF into dma_scatter_add's index layout:
                    # W16[j%16, j//16] = slotF[j%128, j//128]  (j = token row index)
                    trpA = ps_tr.tile([128, 128], dt.float32, space="PSUM", tag="tr")
                    nc.tensor.transpose(out=trpA[0:32, :], in_=slotf[:], identity=idm[:])
                    ts_sb = dp.tile([32, 128], dt.float32, tag="ts_sb")
                    nc.vector.tensor_copy(ts_sb[:], trpA[0:32, :])
                    slotw_f = dp.tile([16, 256], dt.float32, tag="slotw_f")
                    swv = slotw_f[:].rearrange("p (a b) -> p a b", b=8)
                    for ph in range(8):
                        trpB = ps_tr.tile([128, 128], dt.float32, space="PSUM", tag="tr")
                        nc.tensor.transpose(out=trpB[0:16, 0:32],
                                            in_=ts_sb[:, 16 * ph:16 * (ph + 1)],
                                            identity=idm[0:32, 0:32])
                        nc.vector.tensor_copy(swv[:, :, ph], trpB[0:16, 0:32])
                    slotw16 = dp.tile([16, 256], dt.int16, tag="slotw16")
                    nc.vector.tensor_copy(slotw16[:], slotw_f[:])
                    slotw128 = dp.tile([128, 256], dt.int16, tag="slotw128")
                    for g8 in range(8):
                        nc.sync.dma_start(out=slotw128[16 * g8:16 * (g8 + 1), :],
                                          in_=slotw16[:])

                    wpay = wpays[e % 2]
                    wcopy = nc.vector.tensor_copy(wpay[:, :, 1], probs[:, :, e])

                    tab = tables[e % 2]
                    ti = nc.sync.dma_start(
                        out=bass.AP(tab[:].tensor, 0,
                                    [[(TROWS // 128) * PAY, 128],
                                     [1, (TROWS // 128) * PAY]]),
                        in_=tinit[:])
                    for u in prev_tab_users[e % 2]:
                        add_dep_helper(ti.ins, u.ins, reason="table reuse")
                    scat_tab = []
                    for cch in range(4):
                        si = nc.gpsimd.dma_scatter_add(
                            out_ap=tab[:], in_ap=wpay[:, 8 * cch:8 * (cch + 1), :],
                            idxs_ap=slotw128[:, 64 * cch:64 * (cch + 1)],
                            num_idxs=1024, num_idxs_reg=1024, elem_size=PAY, single_packet=False)
                        add_dep_helper(si.ins, ti.ins, reason="scatter after init")
                        add_dep_helper(si.ins, wcopy.ins, reason="scatter after payload")
                        scat_tab.append(si)

                    idxf = dp.tile([16, 80], dt.float32, tag="idxf")
                    r1 = nc.sync.dma_start(
                        out=idxf[:],
                        in_=bass.AP(tab[:].tensor, 0, [[PAY, 16], [16 * PAY, 80]]))
                    idxd = dp.tile([16, 80], dt.float32, tag="idxd")
                    nc.vector.tensor_scalar(out=idxd[:], in0=idxf[:], scalar1=1.0,
                                            scalar2=None, op0=Alu.subtract)
                    idxm = dp.tile([16, 80], dt.float32, tag="idxm")
                    nc.vector.tensor_scalar(out=idxm[:], in0=idxd[:], scalar1=0.0,
                                            scalar2=None, op0=Alu.is_lt)
                    nc.vector.scalar_tensor_tensor(out=idxd[:], in0=idxm[:],
                                                   scalar=float(NT + 1), in1=idxd[:],
                                                   op0=Alu.mult, op1=Alu.add)
                    idx16 = dp.tile([16, 80], dt.int16, tag="idx16")
                    nc.vector.tensor_copy(idx16[:], idxd[:])
                    idx128 = dp.tile([128, 80], dt.int16, tag="idx128")
                    for g8 in range(8):
                        nc.sync.dma_start(out=idx128[16 * g8:16 * (g8 + 1), :], in_=idx16[:])
                    wcol = dp.tile([128, NTILE], dt.float32, tag="wcol")
                    r2 = nc.sync.dma_start(
                        out=wcol[:],
                        in_=bass.AP(tab[:].tensor, 1, [[PAY, 128], [128 * PAY, NTILE]]))
                    for si in scat_tab:
                        add_dep_helper(r1.ins, si.ins, reason="readback after scatter")
                        add_dep_helper(r2.ins, si.ins, reason="readback after scatter")
                    prev_tab_users[e % 2] = [r1, r2]
                    if _DBG and e == 0:
                        dd = nc.sync.dma_start(out=o_dtab[:], in_=tab[:])
                        for si in scat_tab:
                            add_dep_helper(dd.ins, si.ins, reason="dbg")
                        prev_tab_users[e % 2].append(dd)
                        nc.sync.dma_start(out=o_dslw[:], in_=slotw_f[:])
                        idxf32 = dp.tile([128, 80], dt.float32, tag="idxf32")
                        cc = nc.vector.tensor_copy(idxf32[:], idx128[:])
                        nc.sync.dma_start(out=o_didx[:], in_=idxf32[:])

                    # expert weights (bf16)
                    w1_sb = wp.tile([128, 4, H], bf, tag="w1")
                    nc.sync.dma_start(
                        out=w1_sb[:],
                        in_=bass.AP(w1_in[:].tensor, e * D * H,
                                    [[H, 128], [128 * H, 4], [1, H]]))
                    w2_sb = wp.tile([128, 8, D], bf, tag="w2")
                    nc.sync.dma_start(
                        out=w2_sb[:],
                        in_=bass.AP(w2_in[:].tensor, e * H * D,
                                    [[D, 128], [128 * D, 8], [1, D]]))
                    b1_sb = wp.tile([128, 8], dt.float32, tag="b1")
                    nc.sync.dma_start(
                        out=b1_sb[:],
                        in_=bass.AP(b1_in[:].tensor, e * H, [[1, 128], [128, 8]]))
                    b2_row = wp.tile([1, D], bf, tag="b2")
                    nc.sync.dma_start(out=b2_row[:], in_=b2_in[e:e + 1, :])

                    # gather rows (bf16, 512+768 split)
                    xe = dp.tile([128, NTILE, D], bf, tag="xe")
                    if e < 2:
                        nc.vector.memset(xe[:], 0.0)
                    nc.gpsimd.dma_gather(
                        out_ap=xe[:, 0:4, :], in_ap=x_in[:], idxs_ap=idx128[:, 0:32],
                        num_idxs=512, num_idxs_reg=512, elem_size=D, single_packet=False)
                    nc.gpsimd.dma_gather(
                        out_ap=xe[:, 4:NTILE, :], in_ap=x_in[:], idxs_ap=idx128[:, 32:80],
                        num_idxs=CAP - 512, num_idxs_reg=CAP - 512, elem_size=D, single_packet=False)

                    ye = ap_.tile([128, NTILE, D], dt.float32, tag="ye", bufs=1)

                    for (g0, g1) in GROUPS:
                        ng = (g1 - g0) * 128
                        xeT = ap_.tile([128, 4, 512], bf, tag="xeT")
                        for tt in range(g0, g1):
                            for kd in range(4):
                                trp2 = ps_tr.tile([128, 128], bf,
                                                  space="PSUM", tag="trb")
                                nc.tensor.transpose(
                                    out=trp2[:], in_=xe[:, tt, 128 * kd:128 * (kd + 1)],
                                    identity=idm_bf[:])
                                nc.scalar.activation(
                                    xeT[:, kd, 128 * (tt - g0):128 * (tt - g0 + 1)],
                                    trp2[:], ACTF.Copy)
                        hT = ap_.tile([128, 8, 512], bf, tag="hT", bufs=1)
                        for m in range(8):
                            hp = ps_h.tile([128, 512], dt.float32, space="PSUM", tag="hp")
                            for kd in range(4):
                                nc.tensor.matmul(
                                    hp[:, :ng],
                                    lhsT=w1_sb[:, kd, 128 * m:128 * (m + 1)],
                                    rhs=xeT[:, kd, :ng],
                                    start=(kd == 0), stop=(kd == 3))
                            nc.scalar.activation(hT[:, m, :ng], hp[:, :ng], ACTF.Gelu,
                                                 bias=b1_sb[:, m:m + 1])
                        for tt in range(g0, g1):
                            op = ps_o.tile([128, D], dt.float32, space="PSUM", tag="op")
                            for m in range(8):
                                nc.tensor.matmul(
                                    op[:],
                                    lhsT=hT[:, m, 128 * (tt - g0):128 * (tt - g0 + 1)],
                                    rhs=w2_sb[:, m, :],
                                    start=(m == 0), stop=False)
                            nc.tensor.matmul(op[:], lhsT=ones_r[:], rhs=b2_row[:],
                                             start=False, stop=True)
                            nc.scalar.activation(ye[:, tt, :], op[:], ACTF.Copy,
                                                 scale=wcol[:, tt:tt + 1])

                    s1 = nc.gpsimd.dma_scatter_add(
                        out_ap=o_out[:], in_ap=ye[:, 0:4, :], idxs_ap=idx128[:, 0:32],
                        num_idxs=512, num_idxs_reg=512, elem_size=D, single_packet=False)
                    s2 = nc.gpsimd.dma_scatter_add(
                        out_ap=o_out[:], in_ap=ye[:, 4:NTILE, :], idxs_ap=idx128[:, 32:80],
                        num_idxs=CAP - 512, num_idxs_reg=CAP - 512, elem_size=D, single_packet=False)
                    for zi in zero_insts:
                        add_dep_helper(s1.ins, zi.ins, reason="scatter after zero")
                        add_dep_helper(s2.ins, zi.ins, reason="scatter after zero")
                    if prev_scatter:
                        add_dep_helper(s1.ins, prev_scatter[-1].ins, reason="chain")
                    add_dep_helper(s2.ins, s1.ins, reason="chain")
                    prev_scatter.extend([s1, s2])

                nc.sync.dma_start(out=o_cnt[:], in_=counts_f[:])

    nc.compile()
    return nc


def _consts():
    iota8 = np.tile(np.arange(8, dtype=np.float32), NBI)[None, :].repeat(128, 0)
    idxc = (NBI * np.arange(128)[:, None] + np.arange(NBI)[None, :] + 1).astype(np.float32)
    lt = (np.arange(128)[:, None] < np.arange(128)[None, :]).astype(np.float32)
    idm = np.eye(128, dtype=np.float32)
    ones = np.ones((128, 1), np.float32)
    return iota8, idxc, lt, idm, ones


def _stage_inputs(x, router_w, router_b, W1, b1, W2, b2):
    import ml_dtypes
    bfnp = ml_dtypes.bfloat16
    x = np.ascontiguousarray(np.asarray(x, np.float32))
    rw_stage = np.ascontiguousarray(
        np.vstack([np.asarray(router_w, np.float32),
                   np.asarray(router_b, np.float32)[None, :]]))
    W1b = np.ascontiguousarray(np.asarray(W1, np.float32).astype(bfnp))
    W2b = np.ascontiguousarray(np.asarray(W2, np.float32).astype(bfnp))
    b1f = np.ascontiguousarray(np.asarray(b1, np.float32))
    b2b = np.ascontiguousarray(np.asarray(b2, np.float32).astype(bfnp))
    iota8, idxc, lt, idm, ones = _consts()
    j = np.arange(NT)
    perm = NBI * (j % 128) + j // 128
    in_maps = []
    for c in range(N_CORES):
        xs = x[c * NT:(c + 1) * NT]
        x_stage = np.ascontiguousarray(
            np.vstack([xs, np.zeros((1, D), np.float32)]).astype(bfnp))
        xT_stage = np.ascontiguousarray(
            np.vstack([xs.T[:, perm], np.ones((1, NT), np.float32)]))
        in_maps.append({
            "xT_in": xT_stage, "x_in": x_stage, "rw_in": rw_stage,
            "w1_in": W1b, "w2_in": W2b, "b1_in": b1f, "b2_in": b2b,
            "iota8_c": iota8, "idxc_c": idxc, "lt_c": lt, "id_c": idm,
            "ones_c": ones,
        })
    return in_maps


def _postprocess(res):
    out = np.concatenate([res.results[c]["o_out"][:NT] for c in range(N_CORES)], axis=0)
    S = np.zeros(8, np.float64)
    C = np.zeros(8, np.float64)
    for c in range(N_CORES):
        S += res.results[c]["o_aux"][:, 0].astype(np.float64)
        C += res.results[c]["o_cnt"][0].astype(np.float64)
    N = np.float64(N_TOK)
    balance = float((S / N * (C / N)).sum() * E)
    importance = float((S ** 2).mean())
    return out, np.float32(balance + importance)


def kernel(x, router_w, router_b, W1, b1, W2, b2):
    global _COMPILED
    if _COMPILED is None:
        _COMPILED = _build()
    in_maps = _stage_inputs(x, router_w, router_b, W1, b1, W2, b2)
    res = run_bass_kernel_spmd(_COMPILED, in_maps, list(range(N_CORES)))
    return _postprocess(res)


# revision 10
# speedup vs baseline: 1.5456x; 1.5456x over previous
"""MoE (top-2, 8 experts, capacity-factor 1.0) Trainium2 Bass kernel.

Token-parallel across 8 NeuronCores: each core owns a 4096-token shard and
runs router (fp32) -> top-2 -> prefix-sum compaction -> dma_gather dispatch ->
per-expert MLP (bf16 matmuls, fp32 accumulate) -> gated dma_scatter_add
combine, entirely on-device.  Aux-loss partials are reduced on host (8x8
scalars).  Self-contained: hardcodes N=32768, D=512, H=1024, E=8, K=2.
"""

import numpy as np

import concourse.bass as bass
import concourse.bacc as bacc
import concourse.mybir as mybir
import concourse.tile as tile
from concourse.bass_utils import run_bass_kernel_spmd
from concourse.tile_rust import add_dep_helper

dt = mybir.dt
Alu = mybir.AluOpType
ACTF = mybir.ActivationFunctionType
AX = mybir.AxisListType

N_CORES = 8
N_TOK, D, H, E = 32768, 512, 1024, 8
NT = N_TOK // N_CORES          # 4096 tokens per core
NBI = NT // 128                # 32
CAP = 1280                     # per-(core, expert) slot capacity
NTILE = CAP // 128             # 10
PAY = 64                       # table payload in fp32 elems (256 B)
TROWS = 1408
GROUPS = [(0, 4), (4, 8), (8, 10)]

_COMPILED = None


def _build():
    nc = bacc.Bacc("TRN2", target_bir_lowering=False, debug=False,
                   num_devices=N_CORES, enable_asserts=False)

    xT_in = nc.declare_dram_parameter("xT_in", [513, NT], dt.float32, isOutput=False)
    x_in = nc.declare_dram_parameter("x_in", [NT + 1, D], dt.bfloat16, isOutput=False)
    rw_in = nc.declare_dram_parameter("rw_in", [513, 8], dt.float32, isOutput=False)
    w1_in = nc.declare_dram_parameter("w1_in", [E, D, H], dt.bfloat16, isOutput=False)
    w2_in = nc.declare_dram_parameter("w2_in", [E, H, D], dt.bfloat16, isOutput=False)
    b1_in = nc.declare_dram_parameter("b1_in", [E, H], dt.float32, isOutput=False)
    b2_in = nc.declare_dram_parameter("b2_in", [E, D], dt.bfloat16, isOutput=False)
    iota8_c = nc.declare_dram_parameter("iota8_c", [128, NBI * 8], dt.float32, isOutput=False)
    idxc_c = nc.declare_dram_parameter("idxc_c", [128, NBI], dt.float32, isOutput=False)
    lt_c = nc.declare_dram_parameter("lt_c", [128, 128], dt.float32, isOutput=False)
    id_c = nc.declare_dram_parameter("id_c", [128, 128], dt.float32, isOutput=False)
    ones_c = nc.declare_dram_parameter("ones_c", [128, 1], dt.float32, isOutput=False)

    o_out = nc.declare_dram_parameter("o_out", [NT + 1, D], dt.float32, isOutput=True)
    o_aux = nc.declare_dram_parameter("o_aux", [8, 1], dt.float32, isOutput=True)
    o_cnt = nc.declare_dram_parameter("o_cnt", [1, 8], dt.float32, isOutput=True)
    import os as _os
    _DBG = _os.environ.get("KDBG") == "1"
    if _DBG:
        o_dtab = nc.declare_dram_parameter("o_dtab", [TROWS, PAY], dt.float32, isOutput=True)
        o_dslw = nc.declare_dram_parameter("o_dslw", [16, 256], dt.float32, isOutput=True)
        o_didx = nc.declare_dram_parameter("o_didx", [128, 80], dt.float32, isOutput=True)

    bf = dt.bfloat16

    with tile.TileContext(nc) as tc:
        with tc.tile_pool(name="const", bufs=1) as cp, \
             tc.tile_pool(name="route", bufs=1) as rp, \
             tc.tile_pool(name="ps_h", bufs=2, space="PSUM") as ps_h, \
             tc.tile_pool(name="ps_o", bufs=2, space="PSUM") as ps_o, \
             tc.tile_pool(name="ps_tr", bufs=2, space="PSUM") as ps_tr, \
             tc.tile_pool(name="dram", bufs=1, space="DRAM") as dr:

            # ---------- constants ----------
            iota8 = cp.tile([128, NBI * 8], dt.float32)
            idxc = cp.tile([128, NBI], dt.float32)
            ltm = cp.tile([128, 128], dt.float32)
            idm = cp.tile([128, 128], dt.float32)
            ones1 = cp.tile([128, 1], dt.float32)
            ones_r = cp.tile([1, 128], bf)
            nc.vector.memset(ones_r[:], 1.0)
            idm_bf = cp.tile([128, 128], bf)
            nc.sync.dma_start(out=iota8[:], in_=iota8_c[:])
            nc.sync.dma_start(out=idxc[:], in_=idxc_c[:])
            nc.sync.dma_start(out=ltm[:], in_=lt_c[:])
            nc.sync.dma_start(out=idm[:], in_=id_c[:])
            nc.vector.tensor_copy(idm_bf[:], idm[:])
            nc.sync.dma_start(out=ones1[:], in_=ones_c[:])
            rw_sb = cp.tile([128, 4, 8], dt.float32)
            nc.sync.dma_start(
                out=rw_sb[:],
                in_=bass.AP(rw_in[:].tensor, 0, [[8, 128], [128 * 8, 4], [1, 8]]))
            rb_row = cp.tile([1, 8], dt.float32)
            nc.sync.dma_start(out=rb_row[:], in_=rw_in[512:513, :])

            logit = rp.tile([128, NBI, 8], dt.float32)

            # ---------- router (own pool; closes to free 64 KB/part) ----------
            with tc.tile_pool(name="xt", bufs=1) as xp:
                xT_sb = xp.tile([128, 4, NT], dt.float32)
                nc.sync.dma_start(
                    out=xT_sb[:],
                    in_=bass.AP(xT_in[:].tensor, 0, [[NT, 128], [128 * NT, 4], [1, NT]]))
                ones_row = xp.tile([1, NT], dt.float32)
                nc.sync.dma_start(out=ones_row[:], in_=xT_in[512:513, :])
                for g in range(NBI):
                    lp = ps_h.tile([128, 512], dt.float32, space="PSUM", tag="hp")
                    for kd in range(4):
                        nc.tensor.matmul(
                            lp[:, 0:8], lhsT=xT_sb[:, kd, 128 * g:128 * (g + 1)],
                            rhs=rw_sb[:, kd, :], start=(kd == 0), stop=False)
                    nc.tensor.matmul(
                        lp[:, 0:8], lhsT=ones_row[:, 128 * g:128 * (g + 1)],
                        rhs=rb_row[:], start=False, stop=True)
                    nc.scalar.activation(logit[:, g, :], lp[:, 0:8], ACTF.Copy)

            # ---------- softmax ----------
            probs = rp.tile([128, NBI, 8], dt.float32)
            ssum = rp.tile([128, NBI], dt.float32)
            srec = rp.tile([128, NBI], dt.float32)
            nc.scalar.activation(probs[:], logit[:], ACTF.Exp)
            nc.vector.tensor_reduce(ssum[:], probs[:], axis=AX.X, op=Alu.add)
            nc.vector.reciprocal(srec[:], ssum[:])
            nc.vector.tensor_tensor(out=probs[:], in0=probs[:],
                                    in1=srec[:, :, None].to_broadcast([128, NBI, 8]),
                                    op=Alu.mult)

            # ---------- top-2 ----------
            m1 = rp.tile([128, NBI], dt.float32)
            m2 = rp.tile([128, NBI], dt.float32)
            a1 = rp.tile([128, NBI], dt.float32)
            a2 = rp.tile([128, NBI], dt.float32)
            eqs = rp.tile([128, NBI, 8], dt.float32)
            tmp = rp.tile([128, NBI, 8], dt.float32)
            pm = rp.tile([128, NBI, 8], dt.float32)
            iov = iota8[:].rearrange("p (a b) -> p a b", b=8)
            nc.vector.tensor_reduce(m1[:], probs[:], axis=AX.X, op=Alu.max)
            nc.vector.tensor_tensor(out=eqs[:], in0=probs[:],
                                    in1=m1[:, :, None].to_broadcast([128, NBI, 8]),
                                    op=Alu.is_equal)
            nc.vector.tensor_tensor(out=tmp[:], in0=eqs[:], in1=iov, op=Alu.mult)
            nc.vector.tensor_reduce(a1[:], tmp[:], axis=AX.X, op=Alu.add)
            nc.vector.scalar_tensor_tensor(out=pm[:], in0=eqs[:], scalar=-1e9,
                                           in1=probs[:], op0=Alu.mult, op1=Alu.add)
            nc.vector.tensor_reduce(m2[:], pm[:], axis=AX.X, op=Alu.max)
            nc.vector.tensor_tensor(out=eqs[:], in0=pm[:],
                                    in1=m2[:, :, None].to_broadcast([128, NBI, 8]),
                                    op=Alu.is_equal)
            nc.vector.tensor_tensor(out=tmp[:], in0=eqs[:], in1=iov, op=Alu.mult)
            nc.vector.tensor_reduce(a2[:], tmp[:], axis=AX.X, op=Alu.add)

            # ---------- aux partial S_e ----------
            psum_pe = rp.tile([128, 8], dt.float32)
            nc.vector.tensor_reduce(
                psum_pe[:],
                bass.AP(probs[:].tensor, probs[:].offset,
                        [probs[:].ap[0], [1, 8], [8, NBI]]),
                axis=AX.X, op=Alu.add)
            aux_ps = ps_h.tile([128, 512], dt.float32, space="PSUM", tag="hp")
            nc.tensor.matmul(aux_ps[0:8, 0:1], lhsT=psum_pe[:], rhs=ones1[:],
                             start=True, stop=True)
            aux_sb = rp.tile([8, 1], dt.float32)
            nc.vector.tensor_copy(aux_sb[:], aux_ps[0:8, 0:1])
            nc.sync.dma_start(out=o_aux[:], in_=aux_sb[:])

            counts_f = rp.tile([1, 8], dt.float32)

            # ---------- zero output ----------
            zer = rp.tile([128, 2048], dt.float32)
            nc.vector.memset(zer[:], 0.0)
            zero_insts = []
            for c in range(8):
                zv = bass.AP(o_out[:].tensor, c * 128 * 2048, [[2048, 128], [1, 2048]])
                zero_insts.append(nc.sync.dma_start(out=zv, in_=zer[:]))
            zero_insts.append(nc.sync.dma_start(out=o_out[NT:NT + 1, :], in_=zer[0:1, 0:D]))

            # ---------- dispatch + expert compute ----------
            with tc.tile_pool(name="disp", bufs=2) as dp, \
                 tc.tile_pool(name="wts", bufs=2) as wp, \
                 tc.tile_pool(name="act", bufs=2) as ap_:

                wpays = []
                for i in range(2):
                    t = dp.tile([128, NBI, PAY], dt.float32, name=f"wpay{i}",
                                tag=f"wpay{i}", bufs=1)
                    nc.vector.memset(t[:], 0.0)
                    nc.vector.tensor_copy(t[:, :, 0], idxc[:])
                    wpays.append(t)
                tinit = rp.tile([128, (TROWS // 128) * PAY], dt.float32)
                nc.vector.memset(tinit[:], 0.0)
                tables = [dr.tile([TROWS, PAY], dt.float32, name=f"tab{i}",
                                  tag=f"tab{i}", bufs=1) for i in range(2)]
                prev_tab_users = [[], []]
                prev_scatter = []

                for e in range(E):
                    # mask + prefix-sum positions
                    mask = dp.tile([128, NBI], dt.float32, tag="mask")
                    t1 = dp.tile([128, NBI], dt.float32, tag="t1")
                    nc.vector.tensor_scalar(out=mask[:], in0=a1[:], scalar1=float(e),
                                            scalar2=None, op0=Alu.is_equal)
                    nc.vector.tensor_scalar(out=t1[:], in0=a2[:], scalar1=float(e),
                                            scalar2=None, op0=Alu.is_equal)
                    nc.vector.tensor_tensor(out=mask[:], in0=mask[:], in1=t1[:], op=Alu.add)
                    incl = dp.tile([128, NBI], dt.float32, tag="incl")
                    nc.vector.tensor_tensor_scan(out=incl[:], data0=mask[:], data1=mask[:],
                                                 initial=0.0, op0=Alu.add, op1=Alu.bypass)
                    excl = dp.tile([128, NBI], dt.float32, tag="excl")
                    nc.vector.tensor_tensor(out=excl[:], in0=incl[:], in1=mask[:],
                                            op=Alu.subtract)
                    rowp = ps_o.tile([128, D], dt.float32, space="PSUM", tag="op")
                    nc.tensor.matmul(rowp[:, 0:1], lhsT=ltm[:], rhs=incl[:, NBI - 1:NBI],
                                     start=True, stop=True)
                    nc.tensor.matmul(rowp[0:1, 1:2], lhsT=incl[:, NBI - 1:NBI], rhs=ones1[:],
                                     start=True, stop=True)
                    nc.vector.tensor_copy(counts_f[:, e:e + 1], rowp[0:1, 1:2])

                    pos2 = dp.tile([128, NBI], dt.float32, tag="pos2")
                    nc.vector.tensor_scalar(out=pos2[:], in0=excl[:], scalar1=rowp[:, 0:1],
                                            scalar2=-float(CAP), op0=Alu.add, op1=Alu.add)
                    mt = dp.tile([128, NBI], dt.float32, tag="mt")
                    nc.vector.tensor_tensor(out=mt[:], in0=mask[:], in1=pos2[:], op=Alu.mult)
                    slotf = dp.tile([128, NBI], dt.float32, tag="slotf")
                    nc.vector.tensor_scalar(out=slotf[:], in0=mt[:], scalar1=float(CAP),
                                            scalar2=None, op0=Alu.add)

                    # wrap slotF into dma_scatter_add's index layout:
                    # W16[j%16, j//16] = slotF[j%128, j//128]  (j = token row index)
                    trpA = ps_tr.tile([128, 128], dt.float32, space="PSUM", tag="tr")
                    nc.tensor.transpose(out=trpA[0:32, :], in_=slotf[:], identity=idm[:])
                    ts_sb = dp.tile([32, 128], dt.float32, tag="ts_sb")
                    nc.vector.tensor_copy(ts_sb[:], trpA[0:32, :])
                    slotw_f = dp.tile([16, 256], dt.float32, tag="slotw_f")
                    swv = slotw_f[:].rearrange("p (a b) -> p a b", b=8)
                    for ph in range(8):
                        trpB = ps_tr.tile([128, 128], dt.float32, space="PSUM", tag="tr")
                        nc.tensor.transpose(out=trpB[0:16, 0:32],
                                            in_=ts_sb[:, 16 * ph:16 * (ph + 1)],
                                            identity=idm[0:32, 0:32])
                        nc.vector.tensor_copy(swv[:, :, ph], trpB[0:16, 0:32])
                    slotw16 = dp.tile([16, 256], dt.int16, tag="slotw16")
                    nc.vector.tensor_copy(slotw16[:], slotw_f[:])
                    slotw128 = dp.tile([128, 256], dt.int16, tag="slotw128")
                    for g8 in range(8):
                        nc.sync.dma_start(out=slotw128[16 * g8:16 * (g8 + 1), :],
                                          in_=slotw16[:])

                    wpay = wpays[e % 2]
                    wcopy = nc.vector.tensor_copy(wpay[:, :, 1], probs[:, :, e])

                    tab = tables[e % 2]
                    ti = nc.sync.dma_start(
                        out=bass.AP(tab[:].tensor, 0,
                                    [[(TROWS // 128) * PAY, 128],
                                     [1, (TROWS // 128) * PAY]]),
                        in_=tinit[:])
                    for u in prev_tab_users[e % 2]:
                        add_dep_helper(ti.ins, u.ins, reason="table reuse")
                    scat_tab = []
                    for cch in range(4):
                        si = nc.gpsimd.dma_scatter_add(
                            out_ap=tab[:], in_ap=wpay[:, 8 * cch:8 * (cch + 1), :],
                            idxs_ap=slotw128[:, 64 * cch:64 * (cch + 1)],
                            num_idxs=1024, num_idxs_reg=1024, elem_size=PAY)
                        add_dep_helper(si.ins, ti.ins, reason="scatter after init")
                        add_dep_helper(si.ins, wcopy.ins, reason="scatter after payload")
                        scat_tab.append(si)

                    idxf = dp.tile([16, 80], dt.float32, tag="idxf")
                    r1 = nc.sync.dma_start(
                        out=idxf[:],
                        in_=bass.AP(tab[:].tensor, 0, [[PAY, 16], [16 * PAY, 80]]))
                    idxd = dp.tile([16, 80], dt.float32, tag="idxd")
                    nc.vector.tensor_scalar(out=idxd[:], in0=idxf[:], scalar1=1.0,
                                            scalar2=None, op0=Alu.subtract)
                    idxm = dp.tile([16, 80], dt.float32, tag="idxm")
                    nc.vector.tensor_scalar(out=idxm[:], in0=idxd[:], scalar1=0.0,
                                            scalar2=None, op0=Alu.is_lt)
                    nc.vector.scalar_tensor_tensor(out=idxd[:], in0=idxm[:],
                                                   scalar=float(NT + 1), in1=idxd[:],
                                                   op0=Alu.mult, op1=Alu.add)
                    idx16 = dp.tile([16, 80], dt.int16, tag="idx16")
                    nc.vector.tensor_copy(idx16[:], idxd[:])
                    idx128 = dp.tile([128, 80], dt.int16, tag="idx128")
                    for g8 in range(8):
                        nc.sync.dma_start(out=idx128[16 * g8:16 * (g8 + 1), :], in_=idx16[:])
                    wcol = dp.tile([128, NTILE], dt.float32, tag="wcol")
                    r2 = nc.sync.dma_start(
                        out=wcol[:],
                        in_=bass.AP(tab[:].tensor, 1, [[PAY, 128], [128 * PAY, NTILE]]))
                    for si in scat_tab:
                        add_dep_helper(r1.ins, si.ins, reason="readback after scatter")
                        add_dep_helper(r2.ins, si.ins, reason="readback after scatter")
                    prev_tab_users[e % 2] = [r1, r2]
                    if _DBG and e == 0:
                        dd = nc.sync.dma_start(out=o_dtab[:], in_=tab[:])
                        for si in scat_tab:
                            add_dep_helper(dd.ins, si.ins, reason="dbg")
                        prev_tab_users[e % 2].append(dd)
                        nc.sync.dma_start(out=o_dslw[:], in_=slotw_f[:])
                        idxf32 = dp.tile([128, 80], dt.float32, tag="idxf32")
                        cc = nc.vector.tensor_copy(idxf32[:], idx128[:])
                        nc.sync.dma_start(out=o_didx[:], in_=idxf32[:])

                    # expert weights (bf16)
                    w1_sb = wp.tile([128, 4, H], bf, tag="w1")
                    nc.sync.dma_start(
                        out=w1_sb[:],
                        in_=bass.AP(w1_in[:].tensor, e * D * H,
                                    [[H, 128], [128 * H, 4], [1, H]]))
                    w2_sb = wp.tile([128, 8, D], bf, tag="w2")
                    nc.sync.dma_start(
                        out=w2_sb[:],
                        in_=bass.AP(w2_in[:].tensor, e * H * D,
                                    [[D, 128], [128 * D, 8], [1, D]]))
                    b1_sb = wp.tile([128, 8], dt.float32, tag="b1")
                    nc.sync.dma_start(
                        out=b1_sb[:],
                        in_=bass.AP(b1_in[:].tensor, e * H, [[1, 128], [128, 8]]))
                    b2_row = wp.tile([1, D], bf, tag="b2")
                    nc.sync.dma_start(out=b2_row[:], in_=b2_in[e:e + 1, :])

                    # gather rows (bf16, 512+768 split)
                    xe = dp.tile([128, NTILE, D], bf, tag="xe")
                    if e < 2:
                        nc.vector.memset(xe[:], 0.0)
                    nc.gpsimd.dma_gather(
                        out_ap=xe[:, 0:4, :], in_ap=x_in[:], idxs_ap=idx128[:, 0:32],
                        num_idxs=512, num_idxs_reg=512, elem_size=D)
                    nc.gpsimd.dma_gather(
                        out_ap=xe[:, 4:NTILE, :], in_ap=x_in[:], idxs_ap=idx128[:, 32:80],
                        num_idxs=CAP - 512, num_idxs_reg=CAP - 512, elem_size=D)

                    ye = ap_.tile([128, NTILE, D], dt.float32, tag="ye", bufs=1)

                    for (g0, g1) in GROUPS:
                        ng = (g1 - g0) * 128
                        xeT = ap_.tile([128, 4, 512], bf, tag="xeT")
                        for tt in range(g0, g1):
                            for kd in range(4):
                                trp2 = ps_tr.tile([128, 128], bf,
                                                  space="PSUM", tag="trb")
                                nc.tensor.transpose(
                                    out=trp2[:], in_=xe[:, tt, 128 * kd:128 * (kd + 1)],
                                    identity=idm_bf[:])
                                nc.scalar.activation(
                                    xeT[:, kd, 128 * (tt - g0):128 * (tt - g0 + 1)],
                                    trp2[:], ACTF.Copy)
                        hT = ap_.tile([128, 8, 512], bf, tag="hT", bufs=1)
                        for m in range(8):
                            hp = ps_h.tile([128, 512], dt.float32, space="PSUM", tag="hp")
                            for kd in range(4):
                                nc.tensor.matmul(
                                    hp[:, :ng],
                                    lhsT=w1_sb[:, kd, 128 * m:128 * (m + 1)],
                                    rhs=xeT[:, kd, :ng],
                                    start=(kd == 0), stop=(kd == 3))
                            nc.scalar.activation(hT[:, m, :ng], hp[:, :ng], ACTF.Gelu,
                                                 bias=b1_sb[:, m:m + 1])
                        for tt in range(g0, g1):
                            op = ps_o.tile([128, D], dt.float32, space="PSUM", tag="op")
                            for m in range(8):
                                nc.tensor.matmul(
                                    op[:],
                                    lhsT=hT[:, m, 128 * (tt - g0):128 * (tt - g0 + 1)],
                                    rhs=w2_sb[:, m, :],
                                    start=(m == 0), stop=False)
                            nc.tensor.matmul(op[:], lhsT=ones_r[:], rhs=b2_row[:],
                                             start=False, stop=True)
                            nc.scalar.activation(ye[:, tt, :], op[:], ACTF.Copy,
                                                 scale=wcol[:, tt:tt + 1])

                    s1 = nc.gpsimd.dma_scatter_add(
                        out_ap=o_out[:], in_ap=ye[:, 0:4, :], idxs_ap=idx128[:, 0:32],
                        num_idxs=512, num_idxs_reg=512, elem_size=D)
                    s2 = nc.gpsimd.dma_scatter_add(
                        out_ap=o_out[:], in_ap=ye[:, 4:NTILE, :], idxs_ap=idx128[:, 32:80],
                        num_idxs=CAP - 512, num_idxs_reg=CAP - 512, elem_size=D)
                    for zi in zero_insts:
                        add_dep_helper(s1.ins, zi.ins, reason="scatter after zero")
                        add_dep_helper(s2.ins, zi.ins, reason="scatter after zero")
                    if prev_scatter:
                        add_dep_helper(s1.ins, prev_scatter[-1].ins, reason="chain")
                    add_dep_helper(s2.ins, s1.ins, reason="chain")
                    prev_scatter.extend([s1, s2])

                nc.sync.dma_start(out=o_cnt[:], in_=counts_f[:])

    nc.compile()
    return nc


def _consts():
    iota8 = np.tile(np.arange(8, dtype=np.float32), NBI)[None, :].repeat(128, 0)
    idxc = (NBI * np.arange(128)[:, None] + np.arange(NBI)[None, :] + 1).astype(np.float32)
    lt = (np.arange(128)[:, None] < np.arange(128)[None, :]).astype(np.float32)
    idm = np.eye(128, dtype=np.float32)
    ones = np.ones((128, 1), np.float32)
    return iota8, idxc, lt, idm, ones


def _stage_inputs(x, router_w, router_b, W1, b1, W2, b2):
    import ml_dtypes
    bfnp = ml_dtypes.bfloat16
    x = np.ascontiguousarray(np.asarray(x, np.float32))
    rw_stage = np.ascontiguousarray(
        np.vstack([np.asarray(router_w, np.float32),
                   np.asarray(router_b, np.float32)[None, :]]))
    W1b = np.ascontiguousarray(np.asarray(W1, np.float32).astype(bfnp))
    W2b = np.ascontiguousarray(np.asarray(W2, np.float32).astype(bfnp))
    b1f = np.ascontiguousarray(np.asarray(b1, np.float32))
    b2b = np.ascontiguousarray(np.asarray(b2, np.float32).astype(bfnp))
    iota8, idxc, lt, idm, ones = _consts()
    j = np.arange(NT)
    perm = NBI * (j % 128) + j // 128
    in_maps = []
    for c in range(N_CORES):
        xs = x[c * NT:(c + 1) * NT]
        x_stage = np.ascontiguousarray(
            np.vstack([xs, np.zeros((1, D), np.float32)]).astype(bfnp))
        xT_stage = np.ascontiguousarray(
            np.vstack([xs.T[:, perm], np.ones((1, NT), np.float32)]))
        in_maps.append({
            "xT_in": xT_stage, "x_in": x_stage, "rw_in": rw_stage,
            "w1_in": W1b, "w2_in": W2b, "b1_in": b1f, "b2_in": b2b,
            "iota8_c": iota8, "idxc_c": idxc, "lt_c": lt, "id_c": idm,
            "ones_c": ones,
        })
    return in_maps


def _postprocess(res):
    out = np.concatenate([res.results[c]["o_out"][:NT] for c in range(N_CORES)], axis=0)
    S = np.zeros(8, np.float64)
    C = np.zeros(8, np.float64)
    for c in range(N_CORES):
        S += res.results[c]["o_aux"][:, 0].astype(np.float64)
        C += res.results[c]["o_cnt"][0].astype(np.float64)
    N = np.float64(N_TOK)
    balance = float((S / N * (C / N)).sum() * E)
    importance = float((S ** 2).mean())
    return out, np.float32(balance + importance)


def kernel(x, router_w, router_b, W1, b1, W2, b2):
    global _COMPILED
    if _COMPILED is None:
        _COMPILED = _build()
    in_maps = _stage_inputs(x, router_w, router_b, W1, b1, W2, b2)
    res = run_bass_kernel_spmd(_COMPILED, in_maps, list(range(N_CORES)))
    return _postprocess(res)
